# revision 16
# baseline (speedup 1.0000x reference)
"""Trainium2 Bass kernel for nn_BatchProgramCC (gnn_message_passing).

Pipeline (3 NEFF launches):
  K0: TW = emb @ Wc.T + bc  (vocab-sharded over 8 cores, fp16 in/out)
  K1: per-core 256 trees. The host pre-gathers TW[tokens] (a pure
      permutation, no FLOPs) into a partition-major fp16 layout, so the
      device streams contiguous 1MB tiles with 16 large DMAs instead of
      65536 scatter-gather descriptors. Subtree sums via fp16 structure
      matmuls (output transposed to [ch, node]), per-tree max batched 4
      trees per reduce and load-balanced across engines (Act converts 3
      of 4 groups PSUM->fp16 so DVE reduces at 2 elem/cycle; the 4th
      reduces fp32 straight from PSUM) -> relu -> te shard [128, 256].
  K2: bidirectional GRU via Picard fixed-point iteration: instead of
      2048 serial cell evaluations, run 5 parallel sweeps over the whole
      sequence. Each sweep computes gates from the previous h-estimate
      (big matmuls, parallel over t) and then solves the *linear*
      recurrence h_t = z_t h_{t-1} + (1-z_t) n_t exactly with the DVE
      tensor_tensor_scan instruction. Convergence is geometric (~10x
      error reduction per sweep; validated offline vs the exact scan).
      4 fp16 sweeps + fp16 GX precompute reach ~3e-3 (CPU-validated).
      fwd on core 0, bwd on core 1 (host flips the sequence for bwd).

Self-contained: hardcodes all shapes; no sibling imports.
"""

import time

import numpy as np
import ml_dtypes

import concourse.bass as bass
import concourse.mybir as mybir
from concourse import bacc
from concourse.tile import TileContext
from concourse.bass_utils import run_bass_kernel_spmd

F32 = mybir.dt.float32
BF16 = mybir.dt.bfloat16
F16 = mybir.dt.float16
I32 = mybir.dt.int32
BF = ml_dtypes.bfloat16

T_TREES = 2048
P = 256          # nodes per tree
KARY = 4
VOCAB = 30000
E = 128
C = 128
H = 128
NCORES = 8
TREES_PER_CORE = T_TREES // NCORES          # 256
NODES_PER_CORE = TREES_PER_CORE * P         # 65536
VSHARD = VOCAB // NCORES                    # 3750

TREES_PER_GATHER = 8                        # 16 idx cols per indirect DMA
NBATCH = TREES_PER_CORE // TREES_PER_GATHER  # 32

# K2 Picard sweep schedule: dtypes of the GH matmul per sweep.
SWEEPS = ["h16", "h16", "h16", "h16"]
CHUNK = 512
NCHUNK = T_TREES // CHUNK                   # 4

LAST_RESULTS = []   # BassKernelResults stash for test.py profiling
STAGE_WALL = []     # per-stage wall seconds of the run_bass_kernel_spmd calls
_TRACE_KW = {}      # test.py may set {'trace': True}


def _tree_struct():
    """S[i, j] = 1 iff node j is in subtree(i) (including i==j)."""
    pl = np.zeros(P, np.int64)
    for i in range(1, P):
        pl[i] = (i - 1) // KARY
    S = np.zeros((P, P), np.float32)
    for j in range(P):
        a = j
        while True:
            S[a, j] = 1.0
            if a == 0:
                break
            a = int(pl[a])
    return S


# ---------------------------------------------------------------- K0: table
def build_k0():
    nc = bacc.Bacc("TRN2", target_bir_lowering=False, debug=False,
                   num_devices=NCORES)
    embT = nc.dram_tensor("embT", [E, VSHARD], F16, kind="ExternalInput")
    wcT = nc.dram_tensor("wcT", [E, C], F16, kind="ExternalInput")
    bc1 = nc.dram_tensor("bc1", [1, C], F16, kind="ExternalInput")
    tw = nc.dram_tensor("tw", [VSHARD, C], F16, kind="ExternalOutput")

    with TileContext(nc) as tc:
        with (
            tc.tile_pool(name="const", bufs=1) as cp,
            tc.tile_pool(name="work", bufs=4) as wp,
            tc.tile_pool(name="psum", bufs=2, space="PSUM") as pp,
        ):
            wcT_sb = cp.tile([E, C], F16)
            nc.sync.dma_start(out=wcT_sb[:], in_=wcT[:])
            bc_sb = cp.tile([1, C], F16)
            nc.sync.dma_start(out=bc_sb[:], in_=bc1[:])
            ones_sb = cp.tile([1, 128], F16)
            nc.vector.memset(ones_sb[:], 1.0)

            for c0 in range(0, VSHARD, 128):
                m = min(128, VSHARD - c0)
                et = wp.tile([E, 128], F16, tag="et")
                nc.sync.dma_start(out=et[:, :m], in_=embT[:, c0:c0 + m])
                ps = pp.tile([128, C], F32, tag="ps")
                nc.tensor.matmul(
                    out=ps[:m], lhsT=et[:, :m], rhs=wcT_sb[:],
                    start=True, stop=False)
                nc.tensor.matmul(
                    out=ps[:m], lhsT=ones_sb[:, :m], rhs=bc_sb[:],
                    start=False, stop=True)
                ot = wp.tile([128, C], F16, tag="ot")
                nc.vector.tensor_copy(out=ot[:m], in_=ps[:m])
                nc.sync.dma_start(out=tw[c0:c0 + m, :], in_=ot[:m])
    nc.finalize()
    return nc


# ---------------------------------------------------------------- K1: trees
# Host pre-gathers TW[tokens] (pure permutation, no FLOPs) and permutes to
# hr[p, c*128 + f] = TWg[c*128 + p, f] so the device streams contiguous
# tiles with a handful of large DMAs instead of 65536 gather descriptors.
TREES_PER_CHUNK = 16                        # 32 half-tree cols per DMA
NCHUNK_K1 = TREES_PER_CORE // TREES_PER_CHUNK   # 16


def build_k1():
    nc = bacc.Bacc("TRN2", target_bir_lowering=False, debug=False,
                   num_devices=NCORES)
    hr = nc.dram_tensor("hr", [128, (NODES_PER_CORE // 128) * 128], F16,
                        kind="ExternalInput")
    s00t = nc.dram_tensor("s00t", [128, 128], F16, kind="ExternalInput")
    rhi = nc.dram_tensor("rhi", [128, 256], F16, kind="ExternalInput")
    te = nc.dram_tensor("te", [128, TREES_PER_CORE], F32,
                        kind="ExternalOutput")

    KCOL = 2 * TREES_PER_CHUNK              # half-tree cols per chunk

    with TileContext(nc) as tc:
        with (
            tc.tile_pool(name="const", bufs=1) as cp,
            tc.tile_pool(name="gat", bufs=3) as gp,
            tc.tile_pool(name="psum", bufs=2, space="PSUM") as pp,
        ):
            s00t_sb = cp.tile([128, 128], F16)
            nc.sync.dma_start(out=s00t_sb[:], in_=s00t[:])
            rhi_sb = cp.tile([128, 256], F16)
            nc.sync.dma_start(out=rhi_sb[:], in_=rhi[:])
            te_sb = cp.tile([128, TREES_PER_CORE], F32)

            for b in range(NCHUNK_K1):
                g = gp.tile([128, KCOL, 128], F16, tag="g")
                nc.sync.dma_start(
                    out=g[:],
                    in_=hr[:, b * KCOL * 128:(b + 1) * KCOL * 128])
                for j in range(0, TREES_PER_CHUNK, 4):
                    t = b * TREES_PER_CHUNK + j
                    ps = pp.tile([128, 4, 256], F32, tag="ps")
                    for k in range(4):
                        nc.tensor.matmul(out=ps[:, k, :],
                                         lhsT=g[:, 2 * (j + k) + 1, :],
                                         rhs=rhi_sb[:], start=True,
                                         stop=False)
                        nc.tensor.matmul(out=ps[:, k, 0:128],
                                         lhsT=g[:, 2 * (j + k), :],
                                         rhs=s00t_sb[:], start=False,
                                         stop=True)
                    # balance the max-reduce across Act + DVE: Act (idle
                    # otherwise) converts 3 of 4 groups to fp16 so the DVE
                    # reduce runs at 2 elem/cycle; the 4th reduces fp32
                    # straight from PSUM on DVE.
                    if j < 12:
                        c1 = gp.tile([128, 4, 256], F16, tag="cv")
                        nc.scalar.copy(out=c1[:], in_=ps[:])
                        nc.vector.tensor_reduce(
                            out=te_sb[:, t:t + 4], in_=c1[:],
                            axis=mybir.AxisListType.X,
                            op=mybir.AluOpType.max)
                    else:
                        nc.vector.tensor_reduce(
                            out=te_sb[:, t:t + 4], in_=ps[:],
                            axis=mybir.AxisListType.X,
                            op=mybir.AluOpType.max)
            nc.vector.tensor_scalar_max(out=te_sb[:], in0=te_sb[:],
                                        scalar1=0.0)
            nc.sync.dma_start(out=te[:], in_=te_sb[:])
    nc.finalize()
    return nc


# ---------------------------------------------------------------- K2: GRU
def build_k2():
    T = T_TREES
    nc = bacc.Bacc("TRN2", target_bir_lowering=False, debug=False,
                   num_devices=2)
    te = nc.dram_tensor("te", [128, T], F16, kind="ExternalInput")
    whT_b = nc.dram_tensor("whT_b", [128, 384], F16, kind="ExternalInput")
    wiT = nc.dram_tensor("wiT", [128, 384], F16, kind="ExternalInput")
    id_b = nc.dram_tensor("id_b", [128, 128], F16, kind="ExternalInput")
    # brz[:,0] = b_ih_r + b_hh_r, brz[:,1] = b_ih_z + b_hh_z
    brz = nc.dram_tensor("brz", [128, 2], F32, kind="ExternalInput")
    bnih = nc.dram_tensor("bnih", [128, 1], F32, kind="ExternalInput")
    bhn = nc.dram_tensor("bhn", [128, 1], F32, kind="ExternalInput")
    hmax = nc.dram_tensor("hmax", [128, 1], F32, kind="ExternalOutput")

    SIG = mybir.ActivationFunctionType.Sigmoid
    TANH = mybir.ActivationFunctionType.Tanh
    IDENT = mybir.ActivationFunctionType.Identity

    with TileContext(nc) as tc:
        with (
            tc.tile_pool(name="const", bufs=1) as cp,
            tc.tile_pool(name="gates", bufs=3) as sp,
        ):
            te_sb = cp.tile([128, T], F16)
            for c0 in range(0, T, 512):
                nc.sync.dma_start(out=te_sb[:, c0:c0 + 512],
                                  in_=te[:, c0:c0 + 512])
            whb_sb = cp.tile([128, 384], F16)
            nc.sync.dma_start(out=whb_sb[:], in_=whT_b[:])
            wiT_sb = cp.tile([128, 384], F16)
            nc.sync.dma_start(out=wiT_sb[:], in_=wiT[:])
            idb_sb = cp.tile([128, 128], F16)
            nc.sync.dma_start(out=idb_sb[:], in_=id_b[:])
            brz_sb = cp.tile([128, 2], F32)
            nc.sync.dma_start(out=brz_sb[:], in_=brz[:])
            bnih_sb = cp.tile([128, 1], F32)
            nc.sync.dma_start(out=bnih_sb[:], in_=bnih[:])
            bhn_sb = cp.tile([128, 1], F32)
            nc.sync.dma_start(out=bhn_sb[:], in_=bhn[:])

            gxn = cp.tile([128, T], F32)
            gxr_b = cp.tile([128, T], F16)
            gxz_b = cp.tile([128, T], F16)
            hb = cp.tile([128, T + 1], F16)     # h_{t-1} chain
            nc.vector.memset(hb[:, 0:1], 0.0)

            # ---- GX precompute: gx_g = W_ih_g @ x_t + bias_g for all t
            with tc.tile_pool(name="gxp", bufs=3, space="PSUM") as gpp:
                for c in range(NCHUNK):
                    c0 = c * CHUNK
                    sl = slice(c0, c0 + CHUNK)
                    for gi, (dst, bias) in enumerate((
                            (gxr_b, brz_sb[:, 0:1]),
                            (gxz_b, brz_sb[:, 1:2]),
                            (gxn, bnih_sb[:]))):
                        ps = gpp.tile([128, CHUNK], F32, tag="gps")
                        nc.tensor.matmul(
                            out=ps[:],
                            lhsT=wiT_sb[:, gi * 128:(gi + 1) * 128],
                            rhs=te_sb[:, sl], start=True, stop=True)
                        nc.scalar.activation(dst[:, sl], ps[:], IDENT,
                                             bias=bias)

            # ---- Picard sweeps
            psp_cm = tc.tile_pool(name="pstep", bufs=2, space="PSUM")
            psp = psp_cm.__enter__()
            for s, kind in enumerate(SWEEPS):
                first = (s == 0)
                Hbuf, W, ID, GXR, GXZ = hb, whb_sb, idb_sb, gxr_b, gxz_b
                for c in range(NCHUNK):
                    c0 = c * CHUNK
                    sl = slice(c0, c0 + CHUNK)
                    ps_rz = psp.tile([128, 2, CHUNK], F32, tag="psrz")
                    if first:
                        # H == 0: gate args are just gx (+ b_hh_n for n)
                        nc.tensor.matmul(out=ps_rz[:, 0, :], lhsT=ID[:],
                                         rhs=GXR[:, sl],
                                         start=True, stop=True)
                        nc.tensor.matmul(out=ps_rz[:, 1, :], lhsT=ID[:],
                                         rhs=GXZ[:, sl],
                                         start=True, stop=True)
                    else:
                        nc.tensor.matmul(out=ps_rz[:, 0, :], lhsT=W[:, 0:128],
                                         rhs=Hbuf[:, sl],
                                         start=True, stop=False)
                        nc.tensor.matmul(out=ps_rz[:, 0, :], lhsT=ID[:],
                                         rhs=GXR[:, sl],
                                         start=False, stop=True)
                        nc.tensor.matmul(out=ps_rz[:, 1, :], lhsT=W[:, 128:256],
                                         rhs=Hbuf[:, sl],
                                         start=True, stop=False)
                        nc.tensor.matmul(out=ps_rz[:, 1, :], lhsT=ID[:],
                                         rhs=GXZ[:, sl],
                                         start=False, stop=True)
                        ps_n = psp.tile([128, CHUNK], F32, tag="psn")
                        nc.tensor.matmul(out=ps_n[:], lhsT=W[:, 256:384],
                                         rhs=Hbuf[:, sl],
                                         start=True, stop=True)
                    rz_sb = sp.tile([128, 2, CHUNK], F32, tag="rz")
                    nc.scalar.activation(rz_sb[:], ps_rz[:], SIG)
                    r_sb = rz_sb[:, 0, :]
                    z_sb = rz_sb[:, 1, :]
                    u_sb = sp.tile([128, CHUNK], F32, tag="u")
                    if first:
                        # hn = b_hh_n only
                        nc.vector.tensor_scalar_mul(out=u_sb[:], in0=r_sb,
                                                    scalar1=bhn_sb[:])
                    else:
                        t0 = sp.tile([128, CHUNK], F32, tag="t0")
                        nc.vector.tensor_scalar_add(out=t0[:], in0=ps_n[:],
                                                    scalar1=bhn_sb[:])
                        nc.vector.tensor_tensor(out=u_sb[:], in0=r_sb,
                                                in1=t0[:],
                                                op=mybir.AluOpType.mult)
                    v_sb = sp.tile([128, CHUNK], F32, tag="v")
                    nc.gpsimd.tensor_tensor(out=v_sb[:], in0=u_sb[:],
                                            in1=gxn[:, sl],
                                            op=mybir.AluOpType.add)
                    n_sb = sp.tile([128, CHUNK], F32, tag="n")
                    nc.scalar.activation(n_sb[:], v_sb[:], TANH)
                    zc = sp.tile([128, CHUNK], F32, tag="zc")
                    nc.gpsimd.tensor_scalar(out=zc[:], in0=z_sb,
                                            scalar1=-1.0, scalar2=1.0,
                                            op0=mybir.AluOpType.mult,
                                            op1=mybir.AluOpType.add)
                    w_sb = sp.tile([128, CHUNK], F32, tag="w")
                    nc.gpsimd.tensor_tensor(out=w_sb[:], in0=zc[:],
                                            in1=n_sb[:],
                                            op=mybir.AluOpType.mult)
                    nc.vector.tensor_tensor_scan(
                        out=Hbuf[:, c0 + 1:c0 + 1 + CHUNK],
                        data0=z_sb, data1=w_sb[:],
                        initial=Hbuf[:, c0:c0 + 1],
                        op0=mybir.AluOpType.mult, op1=mybir.AluOpType.add)
            psp_cm.__exit__(None, None, None)

            hm = cp.tile([128, 1], F32)
            nc.vector.tensor_reduce(out=hm[:], in_=hb[:, 1:T + 1],
                                    axis=mybir.AxisListType.X,
                                    op=mybir.AluOpType.max)
            nc.sync.dma_start(out=hmax[:], in_=hm[:])
    nc.finalize()
    return nc


_PROGS = {}


def _get(name, builder):
    if name not in _PROGS:
        _PROGS[name] = builder()
    return _PROGS[name]


# ---------------------------------------------------------------- driver
def kernel(tokens, parent, depth, tree_id, emb, Wc, bc,
           w_ih_f, w_hh_f, b_ih_f, b_hh_f,
           w_ih_b, w_hh_b, b_ih_b, b_hh_b, T):
    tokens = np.asarray(tokens).astype(np.int32)
    emb = np.ascontiguousarray(np.asarray(emb), dtype=np.float32)
    Wc = np.asarray(Wc, dtype=np.float32)
    bc = np.asarray(bc, dtype=np.float32)
    LAST_RESULTS.clear()
    STAGE_WALL.clear()

    # ---- K0: TW = emb @ Wc.T + bc, vocab-sharded, bf16
    nc0 = _get("k0", build_k0)
    embT = np.ascontiguousarray(emb.T).astype(np.float16)     # [128, 30000]
    wcT = np.ascontiguousarray(Wc.T).astype(np.float16)  # [128, 128]
    bc1 = bc.reshape(1, C).astype(np.float16)
    in0 = []
    for i in range(NCORES):
        in0.append({
            "embT": np.ascontiguousarray(embT[:, i * VSHARD:(i + 1) * VSHARD]),
            "wcT": wcT,
            "bc1": bc1,
        })
    t0 = time.perf_counter()
    r0 = run_bass_kernel_spmd(nc0, in0, core_ids=list(range(NCORES)),
                              **_TRACE_KW)
    STAGE_WALL.append(time.perf_counter() - t0)
    LAST_RESULTS.append(r0)
    TW = np.concatenate(
        [np.asarray(r0.results[i]["tw"]) for i in range(NCORES)], axis=0)
    TW = np.ascontiguousarray(TW).astype(np.float16, copy=False)

    # ---- K1: tree encodings, tree-sharded
    nc1 = _get("k1", build_k1)
    S = _tree_struct()
    S00T = np.ascontiguousarray(S[0:128, 0:128].T).astype(np.float16)
    RHI = np.ascontiguousarray(
        np.concatenate([S[0:128, 128:256].T, np.eye(128, dtype=np.float32)],
                       axis=1)).astype(np.float16)
    # host-side permutation: gather token rows, lay out partition-major
    TWg = TW[tokens]                                        # [N, 128] bf16
    in1 = []
    for i in range(NCORES):
        hri = TWg[i * NODES_PER_CORE:(i + 1) * NODES_PER_CORE]
        hri = hri.reshape(-1, 128, 128).transpose(1, 0, 2)  # [128, 512, 128]
        hri = np.ascontiguousarray(hri).reshape(128, -1)
        in1.append({"hr": hri, "s00t": S00T, "rhi": RHI})
    t0 = time.perf_counter()
    r1 = run_bass_kernel_spmd(nc1, in1, core_ids=list(range(NCORES)),
                              **_TRACE_KW)
    STAGE_WALL.append(time.perf_counter() - t0)
    LAST_RESULTS.append(r1)
    te = np.concatenate(
        [np.asarray(r1.results[i]["te"], dtype=np.float32)
         for i in range(NCORES)], axis=1)                    # [128, 2048]

    # ---- K2: Picard GRU fwd (core 0) + bwd (core 1)
    nc2 = _get("k2", build_k2)
    ident = np.eye(128, dtype=np.float32)

    def gru_inputs(te_seq, w_ih, w_hh, b_ih, b_hh):
        w_ih = np.asarray(w_ih, np.float32)
        w_hh = np.asarray(w_hh, np.float32)
        b_ih = np.asarray(b_ih, np.float32)
        b_hh = np.asarray(b_hh, np.float32)
        whT = np.concatenate(
            [np.ascontiguousarray(w_hh[g * H:(g + 1) * H].T)
             for g in range(3)], axis=1)
        wiT = np.concatenate(
            [np.ascontiguousarray(w_ih[g * H:(g + 1) * H].T)
             for g in range(3)], axis=1)
        brz = np.stack([
            b_ih[0:128] + b_hh[0:128],
            b_ih[128:256] + b_hh[128:256],
        ], axis=1).astype(np.float32)
        return {
            "te": np.ascontiguousarray(te_seq, dtype=np.float32).astype(
                np.float16),
            "whT_b": whT.astype(np.float16),
            "wiT": wiT.astype(np.float16),
            "id_b": ident.astype(np.float16),
            "brz": brz,
            "bnih": b_ih[256:384].reshape(128, 1).copy(),
            "bhn": b_hh[256:384].reshape(128, 1).copy(),
        }

    in2 = [
        gru_inputs(te, w_ih_f, w_hh_f, b_ih_f, b_hh_f),
        gru_inputs(te[:, ::-1], w_ih_b, w_hh_b, b_ih_b, b_hh_b),
    ]
    t0 = time.perf_counter()
    r2 = run_bass_kernel_spmd(nc2, in2, core_ids=[0, 1], **_TRACE_KW)
    STAGE_WALL.append(time.perf_counter() - t0)
    LAST_RESULTS.append(r2)
    fwd_max = np.asarray(r2.results[0]["hmax"], dtype=np.float32)[:, 0]
    bwd_max = np.asarray(r2.results[1]["hmax"], dtype=np.float32)[:, 0]
    return np.concatenate([fwd_max, bwd_max]).astype(np.float32)
